# revision 8
# baseline (speedup 1.0000x reference)
"""BiMamba Trainium2 kernel, v2 (bf16 full-L phase design).

Sharding: 8 cores = 2 directions x 2 batch x 2 halves of d_inner. Each core
runs an identical SPMD program; per-core differences (direction, batch
element, channel half) are baked into host-prepared inputs.

Per core:
  phase 1 (full L=2048): in-proj -> depthwise conv -> silu -> x-proj ->
    softplus dt -> u = dt*xs, all in bf16 with PE matmuls chunked through a
    single 8-bank PSUM ring.
  phase 2: 16-state selective scan. dA = exp(A*dt) on ACT; dBu = u*B and
    m = h*C elementwise muls split between DVE (2x bf16) and GpSimd;
    the scan itself is DVE tensor_tensor_scan (bf16, full L, no carries).
    B/C rows are broadcast to 128 partitions via DRAM-staged DMA.
  phase 3: y gate + out-proj, partial [128, 2*2048] f32 output;
    host sums the two halves of each (direction, batch) pair.
"""
import numpy as np

try:
    import antenv.axon_hooks  # noqa: F401
except ImportError:
    import sys as _sys
    import types as _types
    _m = _types.ModuleType("antenv.axon_hooks")
    _hh = [None]
    _m.set_axon_ntff_profile_hook = lambda h: _hh.__setitem__(0, h)
    _m.get_axon_ntff_profile_hook = lambda: _hh[0]
    _sys.modules["antenv.axon_hooks"] = _m

import concourse.bacc as bacc
import concourse.tile as tile
from concourse import mybir
from concourse.bass_utils import run_bass_kernel_spmd

f32 = mybir.dt.float32
bf16 = mybir.dt.bfloat16
Alu = mybir.AluOpType
Act = mybir.ActivationFunctionType

CIN = 80
H = 256
DIN = 512
DH = 256      # own scan channels per core
NST = 16
RK = 16
DCONV = 4
B = 2
L = 2048
TC = 512      # PSUM chunk
NCH = L // TC

# states whose dBu/m muls run on GpSimd (rest on DVE 2x bf16)
GPS_STATES = (1, 4, 7, 9, 12, 14)


def _layout16():
    off = {}
    c = 0

    def seg(name, cols):
        nonlocal c
        off[name] = c
        c += cols

    seg("pw", 256)
    for k in range(2):
        seg(f"wixc{k}", 512)
    for k in range(2):
        seg(f"wiz{k}", 256)
    for j in range(4):
        for k in range(DCONV):
            seg(f"cv{j}_{k}", 128)
    for j in range(2):
        seg(f"dg{j}", 128)
    for k in range(4):
        seg(f"wxp{k}", 96)
    seg("wdt", 256)
    for k in range(2):
        seg(f"wo{k}", 256)
    seg("eye", 128)
    return off, c


OFF16, W16COLS = _layout16()


def _layout32():
    off = {}
    c = 0

    def seg(name, cols):
        nonlocal c
        off[name] = c
        c += cols

    for j in range(2):
        seg(f"acol{j}", NST)
    seg("bdt", 2)
    seg("pb", 2)
    seg("cb", 4)
    return off, c


OFF32, W32COLS = _layout32()


def _body(tc_, out, xin, wb, ws, Lv):
    nc = tc_.nc
    from contextlib import ExitStack
    with ExitStack() as ctx:
        pers = ctx.enter_context(tc_.tile_pool(name="pers", bufs=1))
        ph1 = ctx.enter_context(tc_.tile_pool(name="ph1", bufs=1))
        sc2 = ctx.enter_context(tc_.tile_pool(name="sc2", bufs=2))
        bc3 = ctx.enter_context(tc_.tile_pool(name="bc3", bufs=3))
        dstg = ctx.enter_context(tc_.tile_pool(name="dstg", bufs=1, space="DRAM"))
        psA = tc_.alloc_tile_pool(name="psA", bufs=4, space="PSUM")

        xint = pers.tile([CIN, Lv], bf16)
        nc.sync.dma_start(xint[:], xin)
        wbt = pers.tile([128, W16COLS], bf16)
        nc.sync.dma_start(wbt[:], wb)
        wst = pers.tile([128, W32COLS], f32)
        nc.sync.dma_start(wst[:], ws)

        def W(name, p, cols):
            return wbt[0:p, OFF16[name]:OFF16[name] + cols]

        def scol(name, j):
            return wst[0:128, OFF32[name] + j:OFF32[name] + j + 1]

        def psum():
            return psA.tile([128, TC], f32, name="ps", tag="ps")

        # ---------------- phase 1: projections (full L) ----------------
        # in-proj: xp[m] [128, L] bf16
        xp = []
        for m in range(2):
            xpt = ph1.tile([128, Lv], bf16, name=f"xp{m}", tag=f"xp{m}")
            for c in range(NCH):
                ps = psum()
                nc.tensor.matmul(ps[:], W("pw", CIN, 256)[:, 128 * m:128 * (m + 1)],
                                 xint[:, c * TC:(c + 1) * TC], start=True, stop=True)
                nc.scalar.activation(xpt[:, c * TC:(c + 1) * TC], ps[:],
                                     Act.Identity, bias=scol("pb", m))
            xp.append(xpt)

        # W_in(xc) + depthwise conv + silu -> xs[j] [128, L] bf16
        xs = []
        for j in range(4):
            xcc = ph1.tile([128, 4 + Lv], bf16, name=f"xcc{j}", tag=f"xcc{j}")
            nc.vector.memset(xcc[:, 0:4], 0.0)
            for c in range(NCH):
                ps = psum()
                for k in range(2):
                    nc.tensor.matmul(ps[:], W(f"wixc{k}", 128, 512)[:, 128 * j:128 * (j + 1)],
                                     xp[k][:, c * TC:(c + 1) * TC],
                                     start=(k == 0), stop=(k == 1))
                nc.scalar.copy(xcc[:, 4 + c * TC:4 + (c + 1) * TC], ps[:])
            xst = ph1.tile([128, Lv], bf16, name=f"xs{j}", tag=f"xs{j}")
            for c in range(NCH):
                ps = psum()
                for k in range(DCONV):
                    nc.tensor.matmul(ps[:], W(f"cv{j}_{k}", 128, 128),
                                     xcc[:, 1 + c * TC + k:1 + c * TC + k + TC],
                                     start=(k == 0), stop=(k == 3))
                nc.scalar.activation(xst[:, c * TC:(c + 1) * TC], ps[:],
                                     Act.Silu, bias=scol("cb", j))
            xs.append(xst)

        # z gate: g[j] = silu(W_in(z own) @ xp) [128, L] bf16
        g = []
        for j in range(2):
            gt = ph1.tile([128, Lv], bf16, name=f"g{j}", tag=f"g{j}")
            for c in range(NCH):
                ps = psum()
                for k in range(2):
                    nc.tensor.matmul(ps[:], W(f"wiz{k}", 128, 256)[:, 128 * j:128 * (j + 1)],
                                     xp[k][:, c * TC:(c + 1) * TC],
                                     start=(k == 0), stop=(k == 1))
                nc.scalar.activation(gt[:, c * TC:(c + 1) * TC], ps[:], Act.Silu)
            g.append(gt)

        # x-proj: dbl = W_xproj @ xs -> dtR [16,L], BRs/CRs [16,L] bf16
        dtR = ph1.tile([RK, Lv], bf16, name="dtR", tag="dtR")
        BRs = ph1.tile([NST, Lv], bf16, name="BRs", tag="BRs")
        CRs = ph1.tile([NST, Lv], bf16, name="CRs", tag="CRs")
        stg = dstg.tile([2 * NST, Lv], bf16, name="stg", tag="stg")
        for c in range(NCH):
            ps = psum()
            for k in range(4):
                nc.tensor.matmul(ps[0:96, :], W(f"wxp{k}", 128, 96), xs[k][:, c * TC:(c + 1) * TC],
                                 start=(k == 0), stop=(k == 3))
            nc.scalar.copy(dtR[:, c * TC:(c + 1) * TC], ps[0:RK, :])
            nc.scalar.copy(BRs[:, c * TC:(c + 1) * TC], ps[32:32 + NST, :])
            nc.scalar.copy(CRs[:, c * TC:(c + 1) * TC], ps[64:64 + NST, :])
        nc.sync.dma_start(stg[0:NST, :], BRs[:])
        nc.sync.dma_start(stg[NST:2 * NST, :], CRs[:])

        # dt = softplus(W_dt @ dtR + b_dt) -> dtt[j]; u[j] = dt * xs[j]
        dtt, u = [], []
        for j in range(2):
            dj = ph1.tile([128, Lv], bf16, name=f"dtt{j}", tag=f"dtt{j}")
            bc = scol("bdt", j)
            for c in range(NCH):
                ps = psum()
                nc.tensor.matmul(ps[:], W("wdt", RK, 256)[:, 128 * j:128 * (j + 1)],
                                 dtR[:, c * TC:(c + 1) * TC], start=True, stop=True)
                aj = sc2.tile([128, TC], f32, name="aj", tag="aj")
                nc.scalar.activation(aj[:], ps[:], Act.Abs, bias=bc)
                nc.scalar.activation(aj[:], aj[:], Act.Exp, scale=-1.0)
                nc.scalar.activation(aj[:], aj[:], Act.Ln, bias=1.0)
                rj = sc2.tile([128, TC], f32, name="rj", tag="rj")
                nc.scalar.activation(rj[:], ps[:], Act.Relu, bias=bc)
                nc.vector.tensor_tensor(dj[:, c * TC:(c + 1) * TC], aj[:], rj[:], op=Alu.add)
            dtt.append(dj)
            ut = ph1.tile([128, Lv], bf16, name=f"u{j}", tag=f"u{j}")
            nc.vector.tensor_tensor(ut[:], dj[:], xs[j][:], op=Alu.mult)
            u.append(ut)

        # ---------------- phase 2: selective scan over 16 states ----------------
        psA.release()
        psY = tc_.alloc_tile_pool(name="psY", bufs=1, space="PSUM")
        yps = [[None] * NCH for _ in range(2)]
        for j in range(2):
            for c in range(NCH):
                ps = psY.tile([128, TC], f32, name=f"y{j}{c}", tag=f"y{j}{c}")
                nc.tensor.matmul(ps[:], W(f"dg{j}", 128, 128), xs[j][:, c * TC:(c + 1) * TC],
                                 start=True, stop=False, skip_group_check=True)
                yps[j][c] = ps

        for n in range(NST):
            Bsb = bc3.tile([128, Lv], bf16, name="Bsb", tag="Bsb")
            nc.sync.dma_start(Bsb[:], stg[n:n + 1, :].partition_broadcast(128))
            Csb = bc3.tile([128, Lv], bf16, name="Csb", tag="Csb")
            nc.sync.dma_start(Csb[:], stg[NST + n:NST + n + 1, :].partition_broadcast(128))
            eng = nc.gpsimd if n in GPS_STATES else nc.vector
            for j in range(2):
                ac = wst[0:128, OFF32[f"acol{j}"] + n:OFF32[f"acol{j}"] + n + 1]
                dA = sc2.tile([128, Lv], bf16, name=f"dA{j}", tag=f"dA{j}", bufs=2)
                nc.scalar.activation(dA[:], dtt[j][:], Act.Exp, scale=ac)
                dBu = sc2.tile([128, Lv], bf16, name=f"dBu{j}", tag=f"dBu{j}", bufs=3)
                eng.tensor_tensor(dBu[:], u[j][:], Bsb[:], op=Alu.mult)
                h = sc2.tile([128, Lv], bf16, name=f"h{j}", tag=f"h{j}", bufs=2)
                nc.vector.tensor_tensor_scan(h[:], dA[:], dBu[:], 0.0,
                                             op0=Alu.mult, op1=Alu.add)
                m = sc2.tile([128, Lv], bf16, name=f"m{j}", tag=f"m{j}", bufs=2)
                eng.tensor_tensor(m[:], h[:], Csb[:], op=Alu.mult)
                for c in range(NCH):
                    nc.tensor.matmul(yps[j][c][:], W("eye", 128, 128),
                                     m[:, c * TC:(c + 1) * TC],
                                     start=False, stop=(n == NST - 1),
                                     skip_group_check=True)

        # ---------------- phase 3: gate + out-proj ----------------
        yg = []
        for j in range(2):
            ygt = ph1.tile([128, Lv], bf16, name=f"yg{j}", tag=f"yg{j}")
            for c in range(NCH):
                nc.vector.scalar_tensor_tensor(ygt[:, c * TC:(c + 1) * TC],
                                               yps[j][c][:], 1.0,
                                               g[j][:, c * TC:(c + 1) * TC],
                                               op0=Alu.mult, op1=Alu.mult)
            yg.append(ygt)
        psY.release()
        psW = tc_.alloc_tile_pool(name="psW", bufs=2, space="PSUM")
        for m_ in range(2):
            for c in range(NCH):
                ps = psW.tile([128, TC], f32, name="wo", tag="wo")
                for k in range(2):
                    nc.tensor.matmul(ps[:], W(f"wo{k}", 128, 256)[:, 128 * m_:128 * (m_ + 1)],
                                     yg[k][:, c * TC:(c + 1) * TC],
                                     start=(k == 0), stop=(k == 1))
                ot = sc2.tile([128, TC], f32, name="ot", tag="ot")
                nc.scalar.copy(ot[:], ps[:])
                nc.sync.dma_start(out[:, m_ * Lv + c * TC:m_ * Lv + (c + 1) * TC], ot[:])
        psW.release()


def build_program(Lv=L, n_cores=8):
    nc = bacc.Bacc("TRN2", target_bir_lowering=False, debug=False,
                   num_devices=n_cores)
    xin = nc.dram_tensor("xin", [CIN, Lv], bf16, kind="ExternalInput").ap()
    wb = nc.dram_tensor("wb16", [128, W16COLS], bf16, kind="ExternalInput").ap()
    ws = nc.dram_tensor("ws32", [128, W32COLS], f32, kind="ExternalInput").ap()
    out = nc.dram_tensor("out", [128, 2 * Lv], f32, kind="ExternalOutput").ap()
    with tile.TileContext(nc) as tc_:
        _body(tc_, out, xin, wb, ws, Lv)
    nc.compile()
    return nc


def pack_w16(p, half):
    import ml_dtypes
    wb = np.zeros((128, W16COLS), ml_dtypes.bfloat16)
    perm = np.concatenate([np.arange(half * DH, (half + 1) * DH),
                           np.arange((1 - half) * DH, (2 - half) * DH)])

    def put(name, arr):
        r, c = arr.shape
        wb[0:r, OFF16[name]:OFF16[name] + c] = arr.astype(np.float32)

    put("pw", p["proj_w"].T)
    xcW = p["W_in"][:DIN][perm]
    for k in range(2):
        put(f"wixc{k}", xcW.T[128 * k:128 * (k + 1)])
    zW = p["W_in"][DIN + half * DH:DIN + (half + 1) * DH]
    for k in range(2):
        put(f"wiz{k}", zW.T[128 * k:128 * (k + 1)])
    cw = p["conv_w"][perm]
    for j in range(4):
        for k in range(DCONV):
            put(f"cv{j}_{k}", np.diag(cw[128 * j:128 * (j + 1), k]))
    Dp = p["D"][perm][:DH]
    for j in range(2):
        put(f"dg{j}", np.diag(Dp[128 * j:128 * (j + 1)]))
    xpW = p["W_xproj"][:, perm].T
    for k in range(4):
        segm = np.zeros((128, 96), np.float32)
        blkk = xpW[128 * k:128 * (k + 1)]
        segm[:, 0:RK] = blkk[:, 0:RK]
        segm[:, 32:32 + NST] = blkk[:, RK:RK + NST]
        segm[:, 64:64 + NST] = blkk[:, RK + NST:RK + 2 * NST]
        put(f"wxp{k}", segm)
    put("wdt", p["W_dt"][perm][:DH].T)
    woW = p["W_out"][:, perm][:, :DH].T
    for k in range(2):
        put(f"wo{k}", woW[128 * k:128 * (k + 1)])
    put("eye", np.eye(128))
    return wb


def pack_w32(p, half):
    ws = np.zeros((128, W32COLS), np.float32)
    perm = np.concatenate([np.arange(half * DH, (half + 1) * DH),
                           np.arange((1 - half) * DH, (2 - half) * DH)])
    A = -np.exp(p["A_log"])[perm][:DH]
    for j in range(2):
        ws[0:128, OFF32[f"acol{j}"]:OFF32[f"acol{j}"] + NST] = A[128 * j:128 * (j + 1)]
    bdt = p["b_dt"][perm][:DH]
    for j in range(2):
        ws[0:128, OFF32["bdt"] + j] = bdt[128 * j:128 * (j + 1)]
    pb = p["proj_b"]
    for m in range(2):
        ws[0:128, OFF32["pb"] + m] = pb[128 * m:128 * (m + 1)]
    cb = p["conv_b"][perm]
    for j in range(4):
        ws[0:128, OFF32["cb"] + j] = cb[128 * j:128 * (j + 1)]
    return ws


_cache = {}
LAST_RESULTS = None


def kernel(**inputs):
    global LAST_RESULTS
    import ml_dtypes
    if "nc" not in _cache:
        _cache["nc"] = build_program()
    nc = _cache["nc"]

    in_maps = []
    for core in range(8):
        d = core // 4
        b = (core // 2) % 2
        half = core % 2
        pre = "f_" if d == 0 else "b_"
        xv = np.asarray(inputs["x"][b], np.float32)
        if d == 1:
            xv = xv[:, ::-1]
        p = {k: np.asarray(inputs[pre + k], np.float32)
             for k in ("W_in", "conv_w", "conv_b", "W_xproj", "W_dt",
                       "b_dt", "A_log", "D", "W_out")}
        p["proj_w"] = np.asarray(inputs["proj_w"], np.float32)
        p["proj_b"] = np.asarray(inputs["proj_b"], np.float32)
        in_maps.append({"xin": np.ascontiguousarray(xv).astype(ml_dtypes.bfloat16),
                        "wb16": pack_w16(p, half),
                        "ws32": pack_w32(p, half)})
    res = run_bass_kernel_spmd(nc, in_maps, list(range(8)))
    LAST_RESULTS = res
    outs = [r["out"] for r in res.results]
    final = np.empty((B, 2 * H, L), np.float32)
    for b in range(B):
        for d in range(2):
            c0 = d * 4 + b * 2
            s = outs[c0] + outs[c0 + 1]
            final[b, d * H:(d + 1) * H, :] = np.concatenate(
                [s[:, :L], s[:, L:]], axis=0)
    return final


# revision 9
# speedup vs baseline: 163402.1530x; 163402.1530x over previous
"""BiMamba Trainium2 kernel, v2 (bf16 full-L phase design).

Sharding: 8 cores = 2 directions x 2 batch x 2 halves of d_inner. Each core
runs an identical SPMD program; per-core differences (direction, batch
element, channel half) are baked into host-prepared inputs.

Per core:
  phase 1 (full L=2048): in-proj -> depthwise conv -> silu -> x-proj ->
    softplus dt -> u = dt*xs, all in bf16 with PE matmuls chunked through a
    single 8-bank PSUM ring.
  phase 2: 16-state selective scan. dA = exp(A*dt) on ACT; dBu = u*B and
    m = h*C elementwise muls split between DVE (2x bf16) and GpSimd;
    the scan itself is DVE tensor_tensor_scan (bf16, full L, no carries).
    B/C rows are broadcast to 128 partitions via DRAM-staged DMA.
  phase 3: y gate + out-proj, partial [128, 2*2048] f32 output;
    host sums the two halves of each (direction, batch) pair.
"""
import numpy as np

try:
    import antenv.axon_hooks  # noqa: F401
except ImportError:
    import sys as _sys
    import types as _types
    _m = _types.ModuleType("antenv.axon_hooks")
    _hh = [None]
    _m.set_axon_ntff_profile_hook = lambda h: _hh.__setitem__(0, h)
    _m.get_axon_ntff_profile_hook = lambda: _hh[0]
    _sys.modules["antenv.axon_hooks"] = _m

import concourse.bacc as bacc
import concourse.tile as tile
from concourse import mybir
from concourse.bass_utils import run_bass_kernel_spmd

f32 = mybir.dt.float32
bf16 = mybir.dt.bfloat16
Alu = mybir.AluOpType
Act = mybir.ActivationFunctionType

CIN = 80
H = 256
DIN = 512
DH = 256      # own scan channels per core
NST = 16
RK = 16
DCONV = 4
B = 2
L = 2048
TC = 512      # PSUM chunk
NCH = L // TC

# states whose dBu/m muls run on GpSimd (rest on DVE 2x bf16)
GPS_STATES = ()


def _layout16():
    off = {}
    c = 0

    def seg(name, cols):
        nonlocal c
        off[name] = c
        c += cols

    seg("pw", 256)
    for k in range(2):
        seg(f"wixc{k}", 512)
    for k in range(2):
        seg(f"wiz{k}", 256)
    for j in range(4):
        for k in range(DCONV):
            seg(f"cv{j}_{k}", 128)
    for j in range(2):
        seg(f"dg{j}", 128)
    for k in range(4):
        seg(f"wxp{k}", 96)
    seg("wdt", 256)
    for k in range(2):
        seg(f"wo{k}", 256)
    seg("eye", 128)
    return off, c


OFF16, W16COLS = _layout16()


def _layout32():
    off = {}
    c = 0

    def seg(name, cols):
        nonlocal c
        off[name] = c
        c += cols

    for j in range(2):
        seg(f"acol{j}", NST)
    seg("bdt", 2)
    seg("pb", 2)
    seg("cb", 4)
    return off, c


OFF32, W32COLS = _layout32()


def _body(tc_, out, xin, wb, ws, Lv):
    nc = tc_.nc
    from contextlib import ExitStack
    with ExitStack() as ctx:
        pers = ctx.enter_context(tc_.tile_pool(name="pers", bufs=1))
        ph1 = ctx.enter_context(tc_.tile_pool(name="ph1", bufs=1))
        sc2 = ctx.enter_context(tc_.tile_pool(name="sc2", bufs=2))
        bc3 = ctx.enter_context(tc_.tile_pool(name="bc3", bufs=3))
        dstg = ctx.enter_context(tc_.tile_pool(name="dstg", bufs=1, space="DRAM"))
        psA = tc_.alloc_tile_pool(name="psA", bufs=4, space="PSUM")

        xint = pers.tile([CIN, Lv], bf16)
        nc.sync.dma_start(xint[:], xin)
        wbt = pers.tile([128, W16COLS], bf16)
        nc.sync.dma_start(wbt[:], wb)
        wst = pers.tile([128, W32COLS], f32)
        nc.sync.dma_start(wst[:], ws)

        def W(name, p, cols):
            return wbt[0:p, OFF16[name]:OFF16[name] + cols]

        def scol(name, j):
            return wst[0:128, OFF32[name] + j:OFF32[name] + j + 1]

        def psum():
            return psA.tile([128, TC], f32, name="ps", tag="ps")

        # ---------------- phase 1: projections (full L) ----------------
        # in-proj: xp[m] [128, L] bf16
        xp = []
        for m in range(2):
            xpt = ph1.tile([128, Lv], bf16, name=f"xp{m}", tag=f"xp{m}")
            for c in range(NCH):
                ps = psum()
                nc.tensor.matmul(ps[:], W("pw", CIN, 256)[:, 128 * m:128 * (m + 1)],
                                 xint[:, c * TC:(c + 1) * TC], start=True, stop=True)
                nc.scalar.activation(xpt[:, c * TC:(c + 1) * TC], ps[:],
                                     Act.Identity, bias=scol("pb", m))
            xp.append(xpt)

        # W_in(xc) + depthwise conv + silu -> xs[j] [128, L] bf16
        xs = []
        for j in range(4):
            xcc = ph1.tile([128, 4 + Lv], bf16, name=f"xcc{j}", tag=f"xcc{j}")
            nc.vector.memset(xcc[:, 0:4], 0.0)
            for c in range(NCH):
                ps = psum()
                for k in range(2):
                    nc.tensor.matmul(ps[:], W(f"wixc{k}", 128, 512)[:, 128 * j:128 * (j + 1)],
                                     xp[k][:, c * TC:(c + 1) * TC],
                                     start=(k == 0), stop=(k == 1))
                nc.scalar.copy(xcc[:, 4 + c * TC:4 + (c + 1) * TC], ps[:])
            xst = ph1.tile([128, Lv], bf16, name=f"xs{j}", tag=f"xs{j}")
            for c in range(NCH):
                ps = psum()
                for k in range(DCONV):
                    nc.tensor.matmul(ps[:], W(f"cv{j}_{k}", 128, 128),
                                     xcc[:, 1 + c * TC + k:1 + c * TC + k + TC],
                                     start=(k == 0), stop=(k == 3))
                nc.scalar.activation(xst[:, c * TC:(c + 1) * TC], ps[:],
                                     Act.Silu, bias=scol("cb", j))
            xs.append(xst)

        # z gate: g[j] = silu(W_in(z own) @ xp) [128, L] bf16
        g = []
        for j in range(2):
            gt = ph1.tile([128, Lv], bf16, name=f"g{j}", tag=f"g{j}")
            for c in range(NCH):
                ps = psum()
                for k in range(2):
                    nc.tensor.matmul(ps[:], W(f"wiz{k}", 128, 256)[:, 128 * j:128 * (j + 1)],
                                     xp[k][:, c * TC:(c + 1) * TC],
                                     start=(k == 0), stop=(k == 1))
                nc.scalar.activation(gt[:, c * TC:(c + 1) * TC], ps[:], Act.Silu)
            g.append(gt)

        # x-proj: dbl = W_xproj @ xs -> dtR [16,L], BRs/CRs [16,L] bf16
        dtR = ph1.tile([RK, Lv], bf16, name="dtR", tag="dtR")
        BRs = ph1.tile([NST, Lv], bf16, name="BRs", tag="BRs")
        CRs = ph1.tile([NST, Lv], bf16, name="CRs", tag="CRs")
        stg = dstg.tile([2 * NST, Lv], bf16, name="stg", tag="stg")
        for c in range(NCH):
            ps = psum()
            for k in range(4):
                nc.tensor.matmul(ps[0:96, :], W(f"wxp{k}", 128, 96), xs[k][:, c * TC:(c + 1) * TC],
                                 start=(k == 0), stop=(k == 3))
            nc.scalar.copy(dtR[:, c * TC:(c + 1) * TC], ps[0:RK, :])
            nc.scalar.copy(BRs[:, c * TC:(c + 1) * TC], ps[32:32 + NST, :])
            nc.scalar.copy(CRs[:, c * TC:(c + 1) * TC], ps[64:64 + NST, :])
        nc.sync.dma_start(stg[0:NST, :], BRs[:])
        nc.sync.dma_start(stg[NST:2 * NST, :], CRs[:])

        # dt = softplus(W_dt @ dtR + b_dt) -> dtt[j]; u[j] = dt * xs[j]
        dtt, u = [], []
        for j in range(2):
            dj = ph1.tile([128, Lv], bf16, name=f"dtt{j}", tag=f"dtt{j}")
            bc = scol("bdt", j)
            for c in range(NCH):
                ps = psum()
                nc.tensor.matmul(ps[:], W("wdt", RK, 256)[:, 128 * j:128 * (j + 1)],
                                 dtR[:, c * TC:(c + 1) * TC], start=True, stop=True)
                aj = sc2.tile([128, TC], f32, name="aj", tag="aj")
                nc.scalar.activation(aj[:], ps[:], Act.Abs, bias=bc)
                nc.scalar.activation(aj[:], aj[:], Act.Exp, scale=-1.0)
                nc.scalar.activation(aj[:], aj[:], Act.Ln, bias=1.0)
                rj = sc2.tile([128, TC], f32, name="rj", tag="rj")
                nc.scalar.activation(rj[:], ps[:], Act.Relu, bias=bc)
                nc.vector.tensor_tensor(dj[:, c * TC:(c + 1) * TC], aj[:], rj[:], op=Alu.add)
            dtt.append(dj)
            ut = ph1.tile([128, Lv], bf16, name=f"u{j}", tag=f"u{j}")
            nc.vector.tensor_tensor(ut[:], dj[:], xs[j][:], op=Alu.mult)
            u.append(ut)

        # ---------------- phase 2: selective scan over 16 states ----------------
        # split L into two halves so the second half's projections/exp overlap
        # the first half's scan loop; h carried across halves per (n, j).
        HL = Lv // 2
        hcar = pers.tile([128, 2 * NST], bf16, name="hcar", tag="hcar")
        psA.release()
        psY = tc_.alloc_tile_pool(name="psY", bufs=1, space="PSUM")
        yps = [[None] * NCH for _ in range(2)]
        for half in range(2):
            for j in range(2):
                for cc in range(2):
                    c = half * 2 + cc
                    ps = psY.tile([128, TC], f32, name=f"y{j}{c}", tag=f"y{j}{c}")
                    nc.tensor.matmul(ps[:], W(f"dg{j}", 128, 128), xs[j][:, c * TC:(c + 1) * TC],
                                     start=True, stop=False, skip_group_check=True)
                    yps[j][c] = ps
            t0 = half * HL
            for n in range(NST):
                Bsb = bc3.tile([128, HL], bf16, name="Bsb", tag="Bsb")
                nc.sync.dma_start(Bsb[:], stg[n:n + 1, t0:t0 + HL].partition_broadcast(128))
                Csb = bc3.tile([128, HL], bf16, name="Csb", tag="Csb")
                nc.sync.dma_start(Csb[:], stg[NST + n:NST + n + 1, t0:t0 + HL].partition_broadcast(128))
                for j in range(2):
                    ac = wst[0:128, OFF32[f"acol{j}"] + n:OFF32[f"acol{j}"] + n + 1]
                    hc = hcar[:, 2 * n + j:2 * n + j + 1]
                    dA = sc2.tile([128, HL], bf16, name=f"dA{j}", tag=f"dA{j}", bufs=3)
                    nc.scalar.activation(dA[:], dtt[j][:, t0:t0 + HL], Act.Exp, scale=ac)
                    dBu = sc2.tile([128, HL], bf16, name=f"dBu{j}", tag=f"dBu{j}", bufs=3)
                    nc.vector.tensor_tensor(dBu[:], u[j][:, t0:t0 + HL], Bsb[:], op=Alu.mult)
                    h = sc2.tile([128, HL], bf16, name=f"h{j}", tag=f"h{j}", bufs=2)
                    nc.vector.tensor_tensor_scan(h[:], dA[:], dBu[:],
                                                 0.0 if half == 0 else hc,
                                                 op0=Alu.mult, op1=Alu.add)
                    if half == 0:
                        nc.vector.tensor_copy(hc, h[:, HL - 1:HL])
                    m = sc2.tile([128, HL], bf16, name=f"m{j}", tag=f"m{j}", bufs=2)
                    nc.vector.tensor_tensor(m[:], h[:], Csb[:], op=Alu.mult)
                    for cc in range(2):
                        c = half * 2 + cc
                        nc.tensor.matmul(yps[j][c][:], W("eye", 128, 128),
                                         m[:, cc * TC:(cc + 1) * TC],
                                         start=False, stop=(n == NST - 1),
                                         skip_group_check=True)

        # ---------------- phase 3: gate + out-proj ----------------
        yg = []
        for j in range(2):
            ygt = ph1.tile([128, Lv], bf16, name=f"yg{j}", tag=f"yg{j}")
            for c in range(NCH):
                nc.vector.scalar_tensor_tensor(ygt[:, c * TC:(c + 1) * TC],
                                               yps[j][c][:], 1.0,
                                               g[j][:, c * TC:(c + 1) * TC],
                                               op0=Alu.mult, op1=Alu.mult)
            yg.append(ygt)
        psY.release()
        psW = tc_.alloc_tile_pool(name="psW", bufs=2, space="PSUM")
        for m_ in range(2):
            for c in range(NCH):
                ps = psW.tile([128, TC], f32, name="wo", tag="wo")
                for k in range(2):
                    nc.tensor.matmul(ps[:], W(f"wo{k}", 128, 256)[:, 128 * m_:128 * (m_ + 1)],
                                     yg[k][:, c * TC:(c + 1) * TC],
                                     start=(k == 0), stop=(k == 1))
                ot = sc2.tile([128, TC], f32, name="ot", tag="ot")
                nc.scalar.copy(ot[:], ps[:])
                nc.sync.dma_start(out[:, m_ * Lv + c * TC:m_ * Lv + (c + 1) * TC], ot[:])
        psW.release()


def build_program(Lv=L, n_cores=8):
    nc = bacc.Bacc("TRN2", target_bir_lowering=False, debug=False,
                   num_devices=n_cores)
    xin = nc.dram_tensor("xin", [CIN, Lv], bf16, kind="ExternalInput").ap()
    wb = nc.dram_tensor("wb16", [128, W16COLS], bf16, kind="ExternalInput").ap()
    ws = nc.dram_tensor("ws32", [128, W32COLS], f32, kind="ExternalInput").ap()
    out = nc.dram_tensor("out", [128, 2 * Lv], f32, kind="ExternalOutput").ap()
    with tile.TileContext(nc) as tc_:
        _body(tc_, out, xin, wb, ws, Lv)
    nc.compile()
    return nc


def pack_w16(p, half):
    import ml_dtypes
    wb = np.zeros((128, W16COLS), ml_dtypes.bfloat16)
    perm = np.concatenate([np.arange(half * DH, (half + 1) * DH),
                           np.arange((1 - half) * DH, (2 - half) * DH)])

    def put(name, arr):
        r, c = arr.shape
        wb[0:r, OFF16[name]:OFF16[name] + c] = arr.astype(np.float32)

    put("pw", p["proj_w"].T)
    xcW = p["W_in"][:DIN][perm]
    for k in range(2):
        put(f"wixc{k}", xcW.T[128 * k:128 * (k + 1)])
    zW = p["W_in"][DIN + half * DH:DIN + (half + 1) * DH]
    for k in range(2):
        put(f"wiz{k}", zW.T[128 * k:128 * (k + 1)])
    cw = p["conv_w"][perm]
    for j in range(4):
        for k in range(DCONV):
            put(f"cv{j}_{k}", np.diag(cw[128 * j:128 * (j + 1), k]))
    Dp = p["D"][perm][:DH]
    for j in range(2):
        put(f"dg{j}", np.diag(Dp[128 * j:128 * (j + 1)]))
    xpW = p["W_xproj"][:, perm].T
    for k in range(4):
        segm = np.zeros((128, 96), np.float32)
        blkk = xpW[128 * k:128 * (k + 1)]
        segm[:, 0:RK] = blkk[:, 0:RK]
        segm[:, 32:32 + NST] = blkk[:, RK:RK + NST]
        segm[:, 64:64 + NST] = blkk[:, RK + NST:RK + 2 * NST]
        put(f"wxp{k}", segm)
    put("wdt", p["W_dt"][perm][:DH].T)
    woW = p["W_out"][:, perm][:, :DH].T
    for k in range(2):
        put(f"wo{k}", woW[128 * k:128 * (k + 1)])
    put("eye", np.eye(128))
    return wb


def pack_w32(p, half):
    ws = np.zeros((128, W32COLS), np.float32)
    perm = np.concatenate([np.arange(half * DH, (half + 1) * DH),
                           np.arange((1 - half) * DH, (2 - half) * DH)])
    A = -np.exp(p["A_log"])[perm][:DH]
    for j in range(2):
        ws[0:128, OFF32[f"acol{j}"]:OFF32[f"acol{j}"] + NST] = A[128 * j:128 * (j + 1)]
    bdt = p["b_dt"][perm][:DH]
    for j in range(2):
        ws[0:128, OFF32["bdt"] + j] = bdt[128 * j:128 * (j + 1)]
    pb = p["proj_b"]
    for m in range(2):
        ws[0:128, OFF32["pb"] + m] = pb[128 * m:128 * (m + 1)]
    cb = p["conv_b"][perm]
    for j in range(4):
        ws[0:128, OFF32["cb"] + j] = cb[128 * j:128 * (j + 1)]
    return ws


_cache = {}
LAST_RESULTS = None


def kernel(**inputs):
    global LAST_RESULTS
    import ml_dtypes
    if "nc" not in _cache:
        _cache["nc"] = build_program()
    nc = _cache["nc"]

    in_maps = []
    for core in range(8):
        d = core // 4
        b = (core // 2) % 2
        half = core % 2
        pre = "f_" if d == 0 else "b_"
        xv = np.asarray(inputs["x"][b], np.float32)
        if d == 1:
            xv = xv[:, ::-1]
        p = {k: np.asarray(inputs[pre + k], np.float32)
             for k in ("W_in", "conv_w", "conv_b", "W_xproj", "W_dt",
                       "b_dt", "A_log", "D", "W_out")}
        p["proj_w"] = np.asarray(inputs["proj_w"], np.float32)
        p["proj_b"] = np.asarray(inputs["proj_b"], np.float32)
        in_maps.append({"xin": np.ascontiguousarray(xv).astype(ml_dtypes.bfloat16),
                        "wb16": pack_w16(p, half),
                        "ws32": pack_w32(p, half)})
    res = run_bass_kernel_spmd(nc, in_maps, list(range(8)))
    LAST_RESULTS = res
    outs = [r["out"] for r in res.results]
    final = np.empty((B, 2 * H, L), np.float32)
    for b in range(B):
        for d in range(2):
            c0 = d * 4 + b * 2
            s = outs[c0] + outs[c0 + 1]
            final[b, d * H:(d + 1) * H, :] = np.concatenate(
                [s[:, :L], s[:, L:]], axis=0)
    return final
